# revision 17
# baseline (speedup 1.0000x reference)
"""MMoE layer kernel for 8 Trainium2 NeuronCores.

Reference math (B=4096, D=1024, H1=2048, H2=1024, E=7 experts, NS=7 scenes):
  h        = relu(einsum('bd,edh', x, W1) + b1)           # [B,E,H1]
  eo       = relu(einsum('beh,eho', h, W2) + b2)          # [B,E,H2]
  xc       = concat(x, scene_emb[scene])                  # [B, D+16]
  G        = softmax over s of einsum('bd,sde', xc, S)    # [B,E,NS] (after transpose)
  q        = mean_s log(G*7)                              # [B,E]
  score1   = logG[b, e, scene_b]
  select   = drop expert e iff e == argmin_e score1 == argmin_e q
  gate     = softmax_e(G[b,e,scene_b]) * select
  out      = einsum('be,beo', gate, eo); output = stack([out, out])

Sharding: data-parallel over batch (512 rows/core), weights replicated.
Expert MLP matmuls run in bf16 (fp32 accumulation in PSUM); all routing
math stays fp32 so the argmin/select decisions are bit-stable.

Device decomposition of the routing (no cross-partition broadcasts):
  Gpre[b, e*7+s] = x[b] @ Sflat + SE_table[scene_b]   (SE_table = scene_emb @ S[:,D:,:])
  Z = sum_s exp(Gpre); logZ = ln Z; SG = sum_s Gpre
  q      = SG/7 - logZ            (+const, argmin only)
  score1 = sum_s Gpre*onehot_s(scene) - logZ
  gate0  = softmax_e(exp(score1)) (logits in (0,1): no max-subtract needed)
  sel    = 1 - ismin(score1)*ismin(q)
  gate   = gate0 * sel

Schedule (v2): the PE roofline for the expert MLPs is 382us/core; the
optimization budget is everything else.  L1(e0) is the critical path at
boot, so its inputs lead the DMA queue (xtb, then W1[e0] in m-quarters
so the m-major loop can start on the first quarter).  The routing
matmuls (which only gate the L2(e0) evacuation, ~30us later) are moved
INTO the PE stream between L1(e0) and L2(e0), with their PSUM tiles in
the (not-yet-used) L2 pool.  A short warmup covers the ~6us engine-boot
+ ~6us DMA lead so L1 starts with the HAM clock gate already at 8/8.
"""

import sys

if "/opt/trn_rl_repo" not in sys.path:
    sys.path.insert(0, "/opt/trn_rl_repo")

from contextlib import ExitStack

import ml_dtypes
import numpy as np

import concourse.bass as bass
import concourse.tile as tile
from concourse import bacc, mybir
from concourse.bass_utils import run_bass_kernel_spmd

F32 = mybir.dt.float32
BF16 = mybir.dt.bfloat16
AF = mybir.ActivationFunctionType
ALU = mybir.AluOpType
AX = mybir.AxisListType

N_CORES = 8
B, D, H1, H2, E, NS, T = 4096, 1024, 2048, 1024, 7, 7, 2
BL = B // N_CORES          # 512 rows per core
NB = BL // 128             # 4 batch tiles
KT1 = D // 128             # 8  k-tiles, layer 1
MT1 = H1 // 128            # 16 m-tiles, layer 1
KT2 = H1 // 128            # 16 k-tiles, layer 2
NO = H2 // 512             # 2  512-wide out column blocks
EN = E * NS                # 49
NP_BF16 = np.dtype(ml_dtypes.bfloat16)
N_WARMUP = 8               # cold MMs bridge boot->first W1 chunk (+jitter margin)
# W1[e0] DMA chunks along the m (H1) dimension; the m-major L1 loop starts
# after the first 0.5MB chunk lands.
W1E0_CHUNKS = [(0, 256), (256, 512), (512, 1024), (1024, 1536), (1536, 2048)]


def _emit_kernel(tc, aps, has_b1, has_b2):
    nc = tc.nc
    ctx = ExitStack()
    with ctx:
        # Pool stack order matters: the expert-weight pools are allocated
        # BEFORE the routing pool so they never reuse the routing pool's
        # released SBUF addresses — otherwise Tile serializes the first
        # weight DMAs behind every routing matmul (measured 13µs PE stall).
        consts = ctx.enter_context(tc.tile_pool(name="consts", bufs=1))
        w1pool = ctx.enter_context(tc.tile_pool(name="w1", bufs=2))
        w2pool = ctx.enter_context(tc.tile_pool(name="w2", bufs=1))
        htpool = ctx.enter_context(tc.tile_pool(name="ht", bufs=1))
        tmppool = ctx.enter_context(tc.tile_pool(name="tmp", bufs=3))
        l1ps = ctx.enter_context(tc.tile_pool(name="l1ps", bufs=4, space="PSUM"))
        l2ps = ctx.enter_context(tc.tile_pool(name="l2ps", bufs=4, space="PSUM"))
        rpool = tc.alloc_tile_pool(name="routing", bufs=1)

        # ---- PE warm-up: dummy matmuls from a memset tile (no input deps)
        # bridge the engine-boot window (~6us) plus the first-quarter DMA
        # lead so the HAM clock gate is at 8/8 when L1(e0) starts. --------
        warm_sb = rpool.tile([128, 512], BF16)
        nc.vector.memset(warm_sb[:, :], 0.0)
        warm_ps = l1ps.tile([128, 512], F32, tag="ps1", name="warm_ps")
        for _ in range(N_WARMUP):
            nc.tensor.matmul(
                warm_ps[:, :], lhsT=warm_sb[:, 0:128], rhs=warm_sb[:, :],
                start=True, stop=True,
            )

        # ---- DMA queue: each tensor is sequenced to land just before its
        # consumer reaches it in the PE stream.  L1(e0) inputs lead (xtb,
        # then W1[e0] in m-chunks so the m-major loop starts on the first
        # 0.5MB); routing inputs next (consumed from the m11 slot of the
        # L1 loop); W1[e0]'s m12-15 chunk and W2[e0] quarters trail. ------
        # All bulk tensors are pre-transposed HOST-side into the on-chip
        # [128, kt, cols] layout, so every HBM read here is contiguous per
        # partition (8-32KB lines instead of the 0.5-4KB strided lines a
        # device-side rearrange would produce).
        xtb_sb = consts.tile([128, KT1, BL], BF16)
        nc.sync.dma_start(xtb_sb[:, :, :], aps["xTb"][:, :, :])
        w1_e0 = w1pool.tile([128, KT1, H1], BF16, tag="w1")

        def w1e0_chunk_dma(ci):
            lo, hi = W1E0_CHUNKS[ci]
            off = KT1 * lo
            src = aps["w1e0p"][:, off : off + KT1 * (hi - lo)].rearrange(
                "p (t h) -> p t h", t=KT1
            )
            nc.sync.dma_start(w1_e0[:, :, lo:hi], src)

        for ci in range(4):
            w1e0_chunk_dma(ci)
        sflat_sb = rpool.tile([128, KT1, EN], F32)
        nc.sync.dma_start(sflat_sb[:, :, :], aps["sflat"][:, :, :])
        # pack10: [10, 0:49]=SE_table, [10, 49:50]=iota10, [10, 50:562]=scene row
        pack10_sb = rpool.tile([10, EN + 1 + BL], F32)
        nc.sync.dma_start(pack10_sb[:, :], aps["pack10"][:, :])
        # pack53: [128, 0:4]=scene per b-tile, [128, 4:53]=iota7 pattern
        pack53_sb = rpool.tile([128, NB + EN], F32)
        nc.sync.dma_start(pack53_sb[:, :], aps["pack53"][:, :])
        xt_sb = rpool.tile([128, KT1, BL], F32)
        nc.sync.dma_start(xt_sb[:, :, :], aps["xT"][:, :, :])
        w1e0_chunk_dma(4)
        w2_e0 = w2pool.tile([128, KT2, H2], BF16, tag="w2")
        for q in range(4):
            nc.sync.dma_start(
                w2_e0[:, 4 * q : 4 * q + 4, :], aps["w2"][0][:, 4 * q : 4 * q + 4, :]
            )

        # ---- persistent SBUF state --------------------------------------
        gate_sb = consts.tile([128, NB, E], F32)
        acc_sb = consts.tile([128, NB, H2], F32)
        if has_b1:
            b1_sb = consts.tile([128, E * MT1], F32)
            nc.sync.dma_start(b1_sb[:, :], aps["b1t"][:, :])
        if has_b2:
            b2_sb = consts.tile([1, E * H2], BF16)
            nc.sync.dma_start(b2_sb[:, :], aps["b2f"][:, :])
            ones_sb = consts.tile([1, 128], BF16)
            nc.vector.memset(ones_sb[:, :], 1.0)

        # Routing matmul schedule: the 32 Gpre MMs + 4 SE-table MMs are
        # interleaved into the TAIL of L1(e0)'s m-loop (chunks after m=11..
        # 15).  A solid block of these short 49-col matmuls runs the PE at
        # ~60% duty, which trips the HAM activity monitor into re-throttling
        # the clock to 4/8 (measured: 10us of cold L2(e0) in the blocked
        # layout).  Interleaved between 512-col L1 groups the duty stays
        # ~85% and the clock never drops.  PSUM accumulators live in the
        # (not-yet-used) L2 pool so L1's four rotating groups are
        # undisturbed; the open psr groups span the interleave legally
        # (separate banks).
        routing_state = {}

        def routing_prep():
            # onehot over embedding rows, [10, BL]: onehot[r, b] = (scene[b] == r)
            onehot_sb = rpool.tile([10, BL], F32)
            nc.vector.tensor_scalar(
                out=onehot_sb[:, :], in0=pack10_sb[:, EN + 1 :],
                scalar1=pack10_sb[:, EN : EN + 1], scalar2=None, op0=ALU.is_equal,
            )
            routing_state["onehot"] = onehot_sb
            routing_state["psr"] = [
                l2ps.tile([128, EN], F32, tag="ps2", name=f"psr{t}") for t in range(NB)
            ]

        def routing_mm(i):
            kt, t = divmod(i, NB)
            nc.tensor.matmul(
                routing_state["psr"][t][:, :],
                lhsT=xt_sb[:, kt, bass.ts(t, 128)],
                rhs=sflat_sb[:, kt, :],
                start=(kt == 0), stop=False,
            )

        def routing_finish():
            gp = rpool.tile([128, NB * EN], F32)  # all 4 b-tiles side by side
            for t in range(NB):
                nc.tensor.matmul(
                    routing_state["psr"][t][:, :],
                    lhsT=routing_state["onehot"][:, bass.ts(t, 128)],
                    rhs=pack10_sb[:, 0:EN],
                    start=False, stop=True,
                )
                nc.scalar.copy(gp[:, bass.ts(t, EN)], routing_state["psr"][t][:, :])
            return gp

        def routing_chain(gp):
            """Gate computation, fused over all 4 b-tiles ([128, 4*49]).

            Runs on scalar/vector right after the psr->gp copies; the gate
            is only consumed by expert 0's layer-2 evacuation, several
            microseconds later.
            """
            NE = NB * E  # 28
            gp4 = gp.rearrange("p (t e s) -> p (t e) s", s=NS, e=E)
            eex = rpool.tile([128, NB * EN], F32)
            nc.scalar.activation(eex[:, :], gp[:, :], AF.Exp)
            z = rpool.tile([128, NE], F32)
            nc.vector.tensor_reduce(out=z[:, :], in_=eex.rearrange("p (t e s) -> p (t e) s", s=NS, e=E), axis=AX.X, op=ALU.add)
            logz = rpool.tile([128, NE], F32)
            nc.scalar.activation(logz[:, :], z[:, :], AF.Ln)
            sg = rpool.tile([128, NE], F32)
            nc.vector.tensor_reduce(out=sg[:, :], in_=gp4, axis=AX.X, op=ALU.add)
            q = rpool.tile([128, NE], F32)
            nc.vector.scalar_tensor_tensor(
                out=q[:, :], in0=sg[:, :], scalar=1.0 / NS, in1=logz[:, :],
                op0=ALU.mult, op1=ALU.subtract,
            )
            oh = rpool.tile([128, NB * EN], F32)
            for t in range(NB):
                nc.vector.tensor_scalar(
                    out=oh[:, bass.ts(t, EN)], in0=pack53_sb[:, NB:],
                    scalar1=pack53_sb[:, t : t + 1], scalar2=None, op0=ALU.is_equal,
                )
            gsel = rpool.tile([128, NB * EN], F32)
            nc.vector.tensor_tensor(out=gsel[:, :], in0=gp[:, :], in1=oh[:, :], op=ALU.mult)
            s1s = rpool.tile([128, NE], F32)
            nc.vector.tensor_reduce(out=s1s[:, :], in_=gsel.rearrange("p (t e s) -> p (t e) s", s=NS, e=E), axis=AX.X, op=ALU.add)
            score1 = rpool.tile([128, NE], F32)
            nc.vector.tensor_tensor(out=score1[:, :], in0=s1s[:, :], in1=logz[:, :], op=ALU.subtract)

            lg = rpool.tile([128, NE], F32)
            nc.scalar.activation(lg[:, :], score1[:, :], AF.Exp)     # G at scene, in (0,1)
            el = rpool.tile([128, NE], F32)
            nc.scalar.activation(el[:, :], lg[:, :], AF.Exp)         # softmax numerator
            # per-b-tile scalars ([128,1]) for the reductions' broadcasts
            ssum = rpool.tile([128, NB], F32)
            rs = rpool.tile([128, NB], F32)
            m1 = rpool.tile([128, NB], F32)
            m2 = rpool.tile([128, NB], F32)
            k1 = rpool.tile([128, NE], F32)
            k2 = rpool.tile([128, NE], F32)
            g0 = rpool.tile([128, NE], F32)
            el3 = el.rearrange("p (t e) -> p t e", e=E)
            sc3 = score1.rearrange("p (t e) -> p t e", e=E)
            q3 = q.rearrange("p (t e) -> p t e", e=E)
            nc.vector.tensor_reduce(out=ssum[:, :], in_=el3, axis=AX.X, op=ALU.add)
            nc.vector.reciprocal(rs[:, :], ssum[:, :])
            nc.vector.tensor_reduce(out=m1[:, :], in_=sc3, axis=AX.X, op=ALU.min)
            nc.vector.tensor_reduce(out=m2[:, :], in_=q3, axis=AX.X, op=ALU.min)
            for t in range(NB):
                nc.vector.tensor_scalar(
                    out=k1[:, bass.ts(t, E)], in0=score1[:, bass.ts(t, E)],
                    scalar1=m1[:, t : t + 1], scalar2=None, op0=ALU.is_equal,
                )
                nc.vector.tensor_scalar(
                    out=k2[:, bass.ts(t, E)], in0=q[:, bass.ts(t, E)],
                    scalar1=m2[:, t : t + 1], scalar2=None, op0=ALU.is_equal,
                )
                nc.vector.tensor_scalar(
                    out=g0[:, bass.ts(t, E)], in0=el[:, bass.ts(t, E)],
                    scalar1=rs[:, t : t + 1], scalar2=None, op0=ALU.mult,
                )
            kill = rpool.tile([128, NE], F32)
            nc.vector.tensor_tensor(out=kill[:, :], in0=k1[:, :], in1=k2[:, :], op=ALU.mult)
            sel = rpool.tile([128, NE], F32)
            nc.vector.tensor_scalar(
                out=sel[:, :], in0=kill[:, :], scalar1=-1.0, scalar2=1.0,
                op0=ALU.mult, op1=ALU.add,
            )
            gate_flat = gate_sb.rearrange("p t e -> p (t e)")
            nc.vector.tensor_tensor(out=gate_flat[:, :], in0=g0[:, :], in1=sel[:, :], op=ALU.mult)

        # ---- expert MLPs (bf16 matmuls, fp32 accumulation) -------------
        for e in range(E):
            if e == 0:
                w1_sb, w2_sb = w1_e0, w2_e0
            else:
                # All bulk traffic rides the sync HWDGE queue in program
                # order — the per-core DMA fabric saturates at ~350GB/s
                # regardless of queue count, so ordering is what matters.
                # Two half-DMAs per weight: one trigger splits across all 16
                # SDMA engines, and halves complete earlier than one
                # monolithic semaphore.
                w1_sb = w1pool.tile([128, KT1, H1], BF16, tag="w1")
                w1_src = aps["w1"][e]
                nc.sync.dma_start(w1_sb[:, 0 : KT1 // 2, :], w1_src[:, 0 : KT1 // 2, :])
                nc.sync.dma_start(w1_sb[:, KT1 // 2 :, :], w1_src[:, KT1 // 2 :, :])
                w2_sb = w2pool.tile([128, KT2, H2], BF16, tag="w2")
                w2_src = aps["w2"][e]
                nc.sync.dma_start(w2_sb[:, 0 : KT2 // 2, :], w2_src[:, 0 : KT2 // 2, :])
                nc.sync.dma_start(w2_sb[:, KT2 // 2 :, :], w2_src[:, KT2 // 2 :, :])

            # layer 1: hT[f, b] = relu(sum_d W1[d, f] * xT[d, b] + b1[f])
            ht_sb = htpool.tile([128, KT2, BL], BF16, tag="ht")
            for m in range(MT1):
                ps = l1ps.tile([128, BL], F32, tag="ps1")
                for kt in range(KT1):
                    nc.tensor.matmul(
                        ps[:, :],
                        lhsT=w1_sb[:, kt, bass.ts(m, 128)],
                        rhs=xtb_sb[:, kt, :],
                        start=(kt == 0), stop=(kt == KT1 - 1),
                    )
                bias1 = b1_sb[:, e * MT1 + m : e * MT1 + m + 1] if has_b1 else 0.0
                nc.scalar.activation(ht_sb[:, m, :], ps[:, :], AF.Relu, bias=bias1)
                if e == 0 and m >= 11:
                    # interleaved routing chunk (7,7,7,7,4+finish)
                    if m == 11:
                        routing_prep()
                    c = m - 11
                    for i in range(7 * c, min(7 * (c + 1), 32)):
                        routing_mm(i)
                    if m == MT1 - 1:
                        gp = routing_finish()
                        routing_chain(gp)
                        rpool.release()

            # layer 2: out[b, o] = relu(sum_h hT[h, b] * W2[h, o] + b2[o])
            for mb in range(NB):
                for no in range(NO):
                    ps2 = l2ps.tile([128, 512], F32, tag="ps2")
                    for kt in range(KT2):
                        nc.tensor.matmul(
                            ps2[:, :],
                            lhsT=ht_sb[:, kt, bass.ts(mb, 128)],
                            rhs=w2_sb[:, kt, bass.ts(no, 512)],
                            start=(kt == 0),
                            stop=(kt == KT2 - 1 and not has_b2),
                        )
                    if has_b2:
                        nc.tensor.matmul(
                            ps2[:, :],
                            lhsT=ones_sb[:, :],
                            rhs=b2_sb[:, e * H2 + no * 512 : (e * H2 + (no + 1) * 512)],
                            start=False, stop=True,
                        )
                    gcol = gate_sb[:, mb, e : e + 1]
                    if e == 0:
                        nc.scalar.activation(
                            acc_sb[:, mb, bass.ts(no, 512)], ps2[:, :], AF.Relu, scale=gcol
                        )
                    else:
                        tmp = tmppool.tile([128, 512], F32, tag="tmp")
                        nc.scalar.activation(tmp[:, :], ps2[:, :], AF.Relu, scale=gcol)
                        nc.vector.tensor_tensor(
                            out=acc_sb[:, mb, bass.ts(no, 512)],
                            in0=acc_sb[:, mb, bass.ts(no, 512)],
                            in1=tmp[:, :], op=ALU.add,
                        )
                        # Fine-grained output DMA on the last expert so only
                        # one 256KB transfer trails the final matmul.
                        if e == E - 1:
                            nc.sync.dma_start(
                                aps["out"].rearrange("(t p) o -> p t o", p=128)[
                                    :, mb, bass.ts(no, 512)
                                ],
                                acc_sb[:, mb, bass.ts(no, 512)],
                            )


def build(has_b1, has_b2):
    """Build + schedule + compile the Bass program. Returns nc."""
    nc = bacc.Bacc("TRN2", target_bir_lowering=False, debug=False)
    aps = {}
    aps["xT"] = nc.dram_tensor("xT", [128, KT1, BL], F32, kind="ExternalInput").ap()
    aps["xTb"] = nc.dram_tensor("xTb", [128, KT1, BL], BF16, kind="ExternalInput").ap()
    aps["w1"] = nc.dram_tensor("w1", [E, 128, KT1, H1], BF16, kind="ExternalInput").ap()
    aps["w1e0p"] = nc.dram_tensor("w1e0p", [128, KT1 * H1], BF16, kind="ExternalInput").ap()
    aps["w2"] = nc.dram_tensor("w2", [E, 128, KT2, H2], BF16, kind="ExternalInput").ap()
    if has_b1:
        aps["b1t"] = nc.dram_tensor("b1t", [128, E * MT1], F32, kind="ExternalInput").ap()
    if has_b2:
        aps["b2f"] = nc.dram_tensor("b2f", [1, E * H2], BF16, kind="ExternalInput").ap()
    aps["sflat"] = nc.dram_tensor("sflat", [128, KT1, EN], F32, kind="ExternalInput").ap()
    aps["pack10"] = nc.dram_tensor("pack10", [10, EN + 1 + BL], F32, kind="ExternalInput").ap()
    aps["pack53"] = nc.dram_tensor("pack53", [128, NB + EN], F32, kind="ExternalInput").ap()
    aps["out"] = nc.dram_tensor("out", [BL, H2], F32, kind="ExternalOutput").ap()

    with tile.TileContext(nc) as tc:
        _emit_kernel(tc, aps, has_b1, has_b2)
    nc.compile()
    return nc


def make_in_maps(inputs):
    """Host-side layout prep + batch sharding. Returns (in_maps, has_b1, has_b2)."""
    x = np.ascontiguousarray(np.asarray(inputs["x"], dtype=np.float32))
    scene = np.asarray(inputs["scene"]).astype(np.int64)
    W1 = np.asarray(inputs["W1"], dtype=np.float32)
    b1 = np.asarray(inputs["b1"], dtype=np.float32)
    W2 = np.asarray(inputs["W2"], dtype=np.float32)
    b2 = np.asarray(inputs["b2"], dtype=np.float32)
    S = np.asarray(inputs["S"], dtype=np.float32)
    scene_emb = np.asarray(inputs["scene_emb"], dtype=np.float32)

    has_b1 = bool(np.any(b1))
    has_b2 = bool(np.any(b2))

    def to_ptc(a, kt):
        """[kt*128, cols] -> [128, kt, cols] (the on-chip layout, contiguous)."""
        return np.ascontiguousarray(a.reshape(kt, 128, a.shape[-1]).transpose(1, 0, 2))

    w1b = W1.astype(NP_BF16)
    w2b = W2.astype(NP_BF16)
    w1t = np.ascontiguousarray(
        w1b.reshape(E, KT1, 128, H1).transpose(0, 2, 1, 3)
    )                                                        # [E, 128, KT1, H1]
    w1e0p = np.concatenate(
        [w1t[0][:, :, lo:hi].reshape(128, KT1 * (hi - lo)) for lo, hi in W1E0_CHUNKS],
        axis=1,
    )                                                        # chunk-major packing
    w2t = np.ascontiguousarray(
        w2b.reshape(E, KT2, 128, H2).transpose(0, 2, 1, 3)
    )                                                        # [E, 128, KT2, H2]
    sflat = np.ascontiguousarray(S[:, :D, :].transpose(1, 2, 0).reshape(D, EN))
    sett = np.einsum("rm,sme->res", scene_emb, S[:, D:, :]).reshape(scene_emb.shape[0], EN)
    iota7 = np.tile(np.arange(EN, dtype=np.float32) % NS, NB).reshape(1, NB * EN)
    shared = {
        "w1": w1t, "w1e0p": np.ascontiguousarray(w1e0p), "w2": w2t,
        "sflat": to_ptc(sflat, KT1),
    }
    if has_b1:
        shared["b1t"] = np.ascontiguousarray(
            b1.reshape(E, MT1, 128).transpose(2, 0, 1).reshape(128, E * MT1)
        )
    if has_b2:
        shared["b2f"] = np.ascontiguousarray(b2.astype(NP_BF16).reshape(1, E * H2))

    in_maps = []
    for c in range(N_CORES):
        xs = x[c * BL : (c + 1) * BL]
        sc = scene[c * BL : (c + 1) * BL]
        xT = xs.T
        m = dict(shared)
        m["xT"] = to_ptc(xT, KT1)
        m["xTb"] = to_ptc(xT.astype(NP_BF16), KT1)
        # pack10: [10, 0:49]=SE_table, [:, 49]=iota10, [:, 50:]=scene row (x10)
        pack10 = np.zeros((10, EN + 1 + BL), dtype=np.float32)
        pack10[:, :EN] = sett
        pack10[:, EN] = np.arange(10, dtype=np.float32)
        pack10[:, EN + 1 :] = sc.astype(np.float32)[None, :]
        m["pack10"] = np.ascontiguousarray(pack10)
        # pack53: [:, 0:4]=scene per b-tile column, [:, 4:53]=iota7 pattern
        scol = sc.reshape(NB, 128).T.astype(np.float32)          # [128, NB]
        pack53 = np.empty((128, NB + EN), dtype=np.float32)
        pack53[:, :NB] = scol
        pack53[:, NB:] = iota7[0, :EN][None, :]
        m["pack53"] = np.ascontiguousarray(pack53)
        in_maps.append(m)
    return in_maps, has_b1, has_b2


_NC_CACHE = {}


def get_compiled(has_b1, has_b2):
    key = (has_b1, has_b2)
    if key not in _NC_CACHE:
        _NC_CACHE[key] = build(has_b1, has_b2)
    return _NC_CACHE[key]


def run(inputs, trace=False, **kwargs):
    """Run on hardware; returns (full_output, BassKernelResults)."""
    in_maps, has_b1, has_b2 = make_in_maps(inputs)
    nc = get_compiled(has_b1, has_b2)
    res = run_bass_kernel_spmd(nc, in_maps, core_ids=list(range(N_CORES)), trace=trace, **kwargs)
    parts = [res.results[c]["out"] for c in range(N_CORES)]
    out = np.concatenate(parts, axis=0).astype(np.float32)
    full = np.ascontiguousarray(np.broadcast_to(out[None], (T, B, H2)))
    return full, res


def kernel(**inputs):
    full, _ = run(inputs, trace=False)
    return full
